# revision 10
# baseline (speedup 1.0000x reference)
"""Multi-head attention (B=4, S=2048, D=1024, H=16, Hd=64) on 8 trn2 cores.

Sharding: core c = (batch b = c // 2, head-group hg = c % 2). Each core
computes attention for 8 heads of one batch and the corresponding slice of
the output projection; host sums the two partial outputs per batch.

Per-core layout (all matmuls bf16 with fp32 PSUM accumulation):
  xt   = x[b].T                    [D=1024, S=2048]  (lhsT/rhs K-major)
  qT/kT = (Wslice.T @ .. )         [512, 2048]  d-major, 4 pair-tiles of 128
  v    = x @ Wv_slice              [2048, 512]  s-major
  per head: scoresT[k,q] tile = kT.T-block @ qT  -> exp (ScalarE, scale=1/8)
            outT[d,q] += v-block.T @ expT ; denom[q] += ones.T @ expT
  normalize: outT *= broadcast(1/denom) via K=1 ones-matmul
  y = outT.T-blocks @ Wo_slice + bo   [2048, 1024] fp32 partial

Schedule: per exp-iteration the PE has ~0.78us of attention matmuls against
a ~1.1us ScalarE exp, so background work (projections, V, O-proj,
normalize) is chopped into ~2-matmul units and pumped one per iteration
(the Tile scheduler is a readiness-based priority heap; priority =
emission order). Inputs are fetched with one big strided DMA per tensor
(each dma_start costs ~0.6us of SYNC-engine issue time).
"""

import numpy as np
import ml_dtypes

S = 2048
D = 1024
HG_D = 512          # head dims per core (8 heads x 64)
NH = 8              # heads per core
KT = S // 128       # 16 k-tiles
DT = D // 128       # 8 contraction tiles for QKV
ST = S // 128       # 16 s-tiles
OT = HG_D // 128    # 4 contraction tiles for O-proj / pair tiles
N_CORES = 8

BF16 = ml_dtypes.bfloat16

_CACHED_NC = {}


def _build_nc(with_bq=False, with_bk=False, with_bv=False, with_bo=False):
    import concourse.bass as bass  # noqa: F401
    import concourse.mybir as mybir
    import concourse.tile as tile
    from concourse import bacc

    f32 = mybir.dt.float32
    bf16 = mybir.dt.bfloat16
    Exp = mybir.ActivationFunctionType.Exp

    nc = bacc.Bacc("TRN2", target_bir_lowering=False, debug=False,
                   num_devices=N_CORES)

    xt_d = nc.dram_tensor("xt", [D, S], bf16, kind="ExternalInput")
    wq_d = nc.dram_tensor("wq", [D, HG_D], bf16, kind="ExternalInput")
    wk_d = nc.dram_tensor("wk", [D, HG_D], bf16, kind="ExternalInput")
    wv_d = nc.dram_tensor("wv", [D, HG_D], bf16, kind="ExternalInput")
    wo_d = nc.dram_tensor("wo", [HG_D, D], bf16, kind="ExternalInput")
    bq_d = nc.dram_tensor("bqt", [128, OT], f32, kind="ExternalInput")
    bk_d = nc.dram_tensor("bkt", [128, OT], f32, kind="ExternalInput")
    bv_d = nc.dram_tensor("bvr", [1, HG_D], bf16, kind="ExternalInput")
    bo_d = nc.dram_tensor("bor", [1, D], bf16, kind="ExternalInput")
    y_d = nc.dram_tensor("y", [S, D], f32, kind="ExternalOutput")

    with tile.TileContext(nc) as tc:
        with (
            tc.tile_pool(name="cpool", bufs=1) as cpool,
            tc.tile_pool(name="wpool", bufs=2) as wpool,
            tc.tile_pool(name="pspool", bufs=2, space="PSUM") as pspool,
            tc.tile_pool(name="popool", bufs=2, space="PSUM") as popool,
            tc.tile_pool(name="qpool", bufs=2, space="PSUM") as qpool,
        ):
            # ---- persistent SBUF tiles ----
            xt_sb = cpool.tile([128, DT, S], bf16, name="xt_sb")
            wq_sb = cpool.tile([128, DT, HG_D], bf16, name="wq_sb")
            wk_sb = cpool.tile([128, DT, HG_D], bf16, name="wk_sb")
            wv_sb = cpool.tile([128, DT, HG_D], bf16, name="wv_sb")
            wo_sb = cpool.tile([128, OT, D], bf16, name="wo_sb")
            bq_sb = cpool.tile([128, OT], f32, name="bq_sb")
            bk_sb = cpool.tile([128, OT], f32, name="bk_sb")
            bvr_sb = cpool.tile([1, HG_D], bf16, name="bvr_sb")
            bor_sb = cpool.tile([1, D], bf16, name="bor_sb")
            ones_t = cpool.tile([128, 128], bf16, name="ones_t")
            qT_sb = cpool.tile([128, OT, S], bf16, name="qT_sb")
            kT_sb = cpool.tile([128, OT, S], bf16, name="kT_sb")
            # v with a trailing ones column per head: attnv lhsT [128, 65]
            # whose 65th output row accumulates the softmax denominator.
            v_sb = cpool.tile([128, ST, NH, 65], bf16, name="v_sb")
            aoT_sb = cpool.tile([128, OT, S], bf16, name="aoT_sb")
            # persistent denominator staging: junk rows 65..95 memset once
            # so the reciprocal never reads uninitialized SBUF.
            den_sb = [cpool.tile([97, 512], f32, name=f"den{j}")
                      for j in range(2)]

            # ---- DMA: one strided dma_start per tensor (row-block t of
            # the DRAM matrix -> partition p, free (t, c)); xt split by
            # q-column blocks so the first chunks unblock earliest ----
            # Issue the two first-chunk-critical inputs from the (idle)
            # ScalarE DMA queue in parallel with the Sync-queue issues —
            # descriptor generation is ~2.6us/MB serial per queue.
            nc.scalar.dma_start(
                out=xt_sb[:, :, 0:512],
                in_=xt_d[:, 0:512].rearrange("(t p) c -> p t c", p=128))
            nc.scalar.dma_start(
                out=wq_sb[:, :, :],
                in_=wq_d[:, :].rearrange("(t p) c -> p t c", p=128))
            nc.sync.dma_start(
                out=wk_sb[:, :, :],
                in_=wk_d[:, :].rearrange("(t p) c -> p t c", p=128))
            nc.sync.dma_start(
                out=xt_sb[:, :, 512:2048],
                in_=xt_d[:, 512:2048].rearrange("(t p) c -> p t c", p=128))
            nc.sync.dma_start(
                out=wv_sb[:, :, 0:128],
                in_=wv_d[:, 0:128].rearrange("(t p) c -> p t c", p=128))
            nc.sync.dma_start(
                out=wv_sb[:, :, 128:512],
                in_=wv_d[:, 128:512].rearrange("(t p) c -> p t c", p=128))
            nc.sync.dma_start(
                out=wo_sb[:, :, :],
                in_=wo_d[:, :].rearrange("(t p) c -> p t c", p=128))
            nc.sync.dma_start(out=bq_sb[:], in_=bq_d[:])
            nc.sync.dma_start(out=bk_sb[:], in_=bk_d[:])
            nc.sync.dma_start(out=bvr_sb[:], in_=bv_d[:])
            nc.sync.dma_start(out=bor_sb[:], in_=bo_d[:])
            nc.gpsimd.memset(ones_t[:], 1.0)
            nc.vector.memset(v_sb[:, :, :, 64:65], 1.0)
            for j in range(2):
                nc.vector.memset(den_sb[j][64:97, :], 1.0)
            # Warm the PE (HAM un-throttles after ~3.4us of activity) with
            # dummy matmuls while the input DMAs land.
            wtile = qpool.tile([64, 64], f32, tag="pq", name="warm")
            for _ in range(48):
                nc.tensor.matmul(wtile[:], ones_t[0:64, 0:64],
                                 ones_t[0:64, 0:64], start=True, stop=True)

            # ---- background units: each ~2 matmuls (~0.45us PE) ----
            def qk_units(w_sb, b_sb, out_sb, p, jc, with_bias):
                cell = {}

                def mk(k0):
                    def u():
                        if k0 == 0:
                            cell["pq"] = qpool.tile([128, 512], f32,
                                                    tag="pq", name="pq")
                        pq = cell["pq"]
                        for k in range(k0, k0 + 2):
                            nc.tensor.matmul(
                                pq[:],
                                w_sb[:, k, p * 128:(p + 1) * 128],
                                xt_sb[:, k, jc * 512:(jc + 1) * 512],
                                start=(k == 0), stop=(k == DT - 1),
                            )
                        if k0 == DT - 2:
                            dst = out_sb[:, p, jc * 512:(jc + 1) * 512]
                            if with_bias:
                                nc.scalar.add(dst, pq[:], b_sb[:, p:p + 1])
                            else:
                                nc.vector.tensor_copy(dst, pq[:])

                    return u

                return [mk(k0) for k0 in range(0, DT, 2)]

            def v_units(st, c0, c1):
                cell = {}

                def mk(k0):
                    def u():
                        if k0 == 0:
                            cell["pv"] = qpool.tile([128, c1 - c0], f32,
                                                    tag="pq", name="pv")
                        pv = cell["pv"]
                        for k in range(k0, k0 + 2):
                            nc.tensor.matmul(
                                pv[:],
                                xt_sb[:, k, st * 128:(st + 1) * 128],
                                wv_sb[:, k, c0:c1],
                                start=(k == 0),
                                stop=(not with_bv and k == DT - 1),
                            )
                        if k0 == DT - 2:
                            if with_bv:
                                nc.tensor.matmul(
                                    pv[:], ones_t[0:1, 0:128],
                                    bvr_sb[0:1, c0:c1],
                                    start=False, stop=True)
                            nc.vector.tensor_copy(
                                v_sb[:, st, c0 // 64:c1 // 64, 0:64],
                                pv.rearrange("p (h c) -> p h c", c=64))

                    return u

                return [mk(k0) for k0 in range(0, DT, 2)]

            pending = {c: [] for c in range(4)}

            def flush_unit(c):
                def u():
                    off2, p2, jb2, rb2, row = pending[c].pop(0)
                    off3, p3, jb3, rb3, row3 = pending[c].pop(0)
                    assert p2 == p3 and jb2 == jb3
                    bt = qpool.tile([128, 512], f32, tag="pq", name="bt")
                    nc.tensor.matmul(
                        bt[off2:off2 + 64, :],
                        ones_t[row:row + 1, 0:64],
                        rb2[row:row + 1, :],
                        start=True, stop=True,
                    )
                    nc.tensor.matmul(
                        bt[off3:off3 + 64, :],
                        ones_t[row3:row3 + 1, 0:64],
                        rb3[row3:row3 + 1, :],
                        start=True, stop=True,
                    )
                    nc.vector.tensor_mul(
                        aoT_sb[:, p2, jb2], aoT_sb[:, p2, jb2], bt[:])

                return [u]

            def oproj_units(st):
                cell = {}

                def mk(l, k0):
                    def u():
                        if l == 0 and k0 == 0:
                            cell["yt"] = wpool.tile([128, D], f32, tag="y",
                                                    bufs=2, name="yt")
                        if k0 == 0:
                            cell["py"] = qpool.tile([128, 512], f32,
                                                    tag="pq", name="py")
                        yt, py = cell["yt"], cell["py"]
                        for kt in range(k0, k0 + 2):
                            nc.tensor.matmul(
                                py[:],
                                aoT_sb[:, kt, st * 128:(st + 1) * 128],
                                wo_sb[:, kt, l * 512:(l + 1) * 512],
                                start=(kt == 0),
                                stop=(not with_bo and kt == OT - 1),
                            )
                        if k0 == OT - 2:
                            if with_bo:
                                nc.tensor.matmul(
                                    py[:], ones_t[0:1, 0:128],
                                    bor_sb[0:1, l * 512:(l + 1) * 512],
                                    start=False, stop=True)
                            nc.vector.tensor_copy(
                                yt[:, l * 512:(l + 1) * 512], py[:])
                            if l == 1:
                                nc.sync.dma_start(
                                    out=y_d[st * 128:(st + 1) * 128, :],
                                    in_=yt[:, :])

                    return u

                return [mk(0, 0), mk(0, 2), mk(1, 0), mk(1, 2)]

            bg = []

            def pump(n):
                for _ in range(n):
                    if bg:
                        bg.pop(0)()

            chunk_seq = [0]

            # ---- attention chunk (q-chunk c, head-pair p) ----
            # Both heads of a pair are processed together so consecutive
            # scores matmuls alternate PE row halves (rows 0-63 / 64-127),
            # which lets the hardware overlap them (~2x scores throughput).
            def emit_chunk(c, p, inline=None):
                jb = slice(c * 512, (c + 1) * 512)
                otA = popool.tile([65, 512], f32, tag="po", name="otA")
                otB = popool.tile([65, 512], f32, tag="po", name="otB")
                for i in range(KT):
                    if inline is not None:
                        for u in inline(i):
                            u()
                    stt = pspool.tile([128, 1024], f32, tag="ps", name="stt")
                    for off in (0, 64):
                        nc.tensor.matmul(
                            stt[:, off * 8:off * 8 + 512],
                            kT_sb[off:off + 64, p, i * 128:(i + 1) * 128],
                            qT_sb[off:off + 64, p, jb],
                            start=True, stop=True,
                        )
                    et = wpool.tile([128, 1024], bf16, tag="exp", bufs=4,
                                    name="et")
                    nc.scalar.activation(et[:], stt[:], Exp, scale=0.125)
                    for ot, hh in ((otA, 0), (otB, 1)):
                        nc.tensor.matmul(
                            ot[:],
                            v_sb[:, i, 2 * p + hh, :],
                            et[:, hh * 512:(hh + 1) * 512],
                            start=(i == 0), stop=(i == KT - 1),
                        )
                    pump(2 if 1 <= i <= 4 else 1)
                # Drain attn rows (cross-partition for the odd head) fast so
                # the PSUM tiles free, then gather the denominators to rows
                # 64 (A) / 96 (B) of the staging tile and reciprocal both.
                nc.vector.tensor_copy(aoT_sb[0:64, p, jb], otA[0:64, :])
                nc.vector.tensor_copy(aoT_sb[64:128, p, jb], otB[0:64, :])
                den = den_sb[chunk_seq[0] % 2]
                chunk_seq[0] += 1
                nc.vector.tensor_copy(den[64:65, :], otA[64:65, :])
                nc.vector.tensor_copy(den[96:97, :], otB[64:65, :])
                rf = wpool.tile([97, 512], f32, tag="rf", name="rf")
                rb = wpool.tile([65, 512], bf16, tag="rb", bufs=10, name="rb")
                nc.vector.reciprocal(rf[64:97, :], den[64:97, :])
                nc.vector.tensor_copy(rb[64:65, :], rf[64:65, :])
                nc.vector.tensor_copy(rb[32:33, :], rf[96:97, :])
                pending[c].append((0, p, jb, rb, 64))
                pending[c].append((64, p, jb, rb, 32))

            # ---- emission schedule (4 q-chunk groups, head-pair inner) ----
            # Head: minimal deps for chunk (c0, p0).
            for u in qk_units(wk_sb, bk_sb, kT_sb, 0, 0, with_bk):
                u()
            for u in qk_units(wq_sb, bq_sb, qT_sb, 0, 0, with_bq):
                u()

            def queue_qk(w, b, o, p, jc, wb):
                bg.extend(qk_units(w, b, o, p, jc, wb))

            # inline generators for the two v-heavy chunks of c0:
            # (c0,p0) computes v head-pair 0 (cols 0-128) + its own kT tail;
            # (c0,p1) computes v cols 128-512.
            kt_tail = []
            for jc in range(1, 4):
                kt_tail.extend(qk_units(wk_sb, bk_sb, kT_sb, 0, jc, with_bk))

            def inline_c0p0(i):
                us = v_units(i, 0, 128)[:]  # 4 units, ~0.45us each
                take, kt_rest = kt_tail[:1], kt_tail[1:]
                kt_tail[:] = kt_rest
                return us[0:2] + take + us[2:4]

            def inline_c0p1(i):
                return v_units(i, 128, 512)

            for c in range(4):
                for p in range(4):
                    inline = None
                    if c == 0:
                        if p == 0:
                            inline = inline_c0p0
                            queue_qk(wk_sb, bk_sb, kT_sb, 1, 0, with_bk)
                            queue_qk(wq_sb, bq_sb, qT_sb, 1, 0, with_bq)
                            for jc in range(1, 4):
                                queue_qk(wk_sb, bk_sb, kT_sb, 1, jc, with_bk)
                        elif p == 1:
                            inline = inline_c0p1
                            for jc in range(4):
                                queue_qk(wk_sb, bk_sb, kT_sb, 2, jc, with_bk)
                            queue_qk(wq_sb, bq_sb, qT_sb, 2, 0, with_bq)
                        elif p == 2:
                            for jc in range(4):
                                queue_qk(wk_sb, bk_sb, kT_sb, 3, jc, with_bk)
                            queue_qk(wq_sb, bq_sb, qT_sb, 3, 0, with_bq)
                        else:
                            for pp in range(4):
                                queue_qk(wq_sb, bq_sb, qT_sb, pp, 1, with_bq)
                    else:
                        if p == 0:
                            for _ in range(4):
                                bg.extend(flush_unit(c - 1))
                            bg.extend(oproj_units(4 * (c - 1) + 0))
                            bg.extend(oproj_units(4 * (c - 1) + 1))
                        elif p == 1:
                            bg.extend(oproj_units(4 * (c - 1) + 2))
                            bg.extend(oproj_units(4 * (c - 1) + 3))
                            if c == 3:
                                # flush c3's chunks as soon as each lands so
                                # only the last chunk's normalize is in the
                                # tail chain
                                bg.extend(flush_unit(3))
                        elif p == 2:
                            if c < 3:
                                for pp in range(4):
                                    queue_qk(wq_sb, bq_sb, qT_sb, pp, c + 1,
                                             with_bq)
                            else:
                                bg.extend(flush_unit(3))
                        elif p == 3 and c == 3:
                            bg.extend(flush_unit(3))
                    emit_chunk(c, p, inline=inline)
            # tail: last chunk's normalize + last group's output projection
            pump(len(bg))
            flush_unit(3)[0]()
            for st in range(12, 16):
                for u in oproj_units(st):
                    u()

    nc.compile()
    return nc


def get_nc(with_bq=False, with_bk=False, with_bv=False, with_bo=False):
    key = (with_bq, with_bk, with_bv, with_bo)
    if key not in _CACHED_NC:
        _CACHED_NC[key] = _build_nc(*key)
    return _CACHED_NC[key]


def make_in_maps(x, Wq, bq, Wk, bk, Wv, bv, Wo, bo):
    x = np.asarray(x, dtype=np.float32)
    in_maps = []
    for c in range(N_CORES):
        b, hg = c // 2, c % 2
        sl = slice(hg * HG_D, (hg + 1) * HG_D)
        in_maps.append({
            "xt": np.ascontiguousarray(np.asarray(x[b]).T).astype(BF16),
            "wq": np.ascontiguousarray(np.asarray(Wq)[:, sl]).astype(BF16),
            "wk": np.ascontiguousarray(np.asarray(Wk)[:, sl]).astype(BF16),
            "wv": np.ascontiguousarray(np.asarray(Wv)[:, sl]).astype(BF16),
            "wo": np.ascontiguousarray(np.asarray(Wo)[sl, :]).astype(BF16),
            "bqt": np.ascontiguousarray(
                np.asarray(bq, np.float32)[sl].reshape(OT, 128).T),
            "bkt": np.ascontiguousarray(
                np.asarray(bk, np.float32)[sl].reshape(OT, 128).T),
            "bvr": np.asarray(bv, np.float32)[sl].reshape(1, HG_D).astype(BF16),
            "bor": (np.asarray(bo, np.float32) if hg == 0
                    else np.zeros(D, np.float32)).reshape(1, D).astype(BF16),
        })
    return in_maps


def bias_flags(bq, bk, bv, bo):
    return {
        "with_bq": bool(np.any(np.asarray(bq))),
        "with_bk": bool(np.any(np.asarray(bk))),
        "with_bv": bool(np.any(np.asarray(bv))),
        "with_bo": bool(np.any(np.asarray(bo))),
    }


def run_cores(in_maps, trace=False, **flags):
    try:
        import ntff_shim
        ntff_shim.install()
    except Exception:
        pass
    from concourse.bass_utils import run_bass_kernel_spmd

    nc = get_nc(**flags)
    return run_bass_kernel_spmd(nc, in_maps, list(range(N_CORES)), trace=trace)


def combine(results):
    y = np.empty((4, S, D), np.float32)
    for b in range(4):
        y[b] = results[2 * b]["y"] + results[2 * b + 1]["y"]
    return y


def kernel(x, Wq, bq, Wk, bk, Wv, bv, Wo, bo):
    in_maps = make_in_maps(x, Wq, bq, Wk, bk, Wv, bv, Wo, bo)
    flags = bias_flags(bq, bk, bv, bo)
    res = run_cores(in_maps, trace=False, **flags)
    return combine(res.results)
